# revision 1
# baseline (speedup 1.0000x reference)
"""Trainium2 Bass kernel for nn_Attn_loc_distance (embedding lookup).

reference:
    idx = venueid2coor[inputs_poi]            # [B,S]   (B=64, S=100)
    d   = poi_distance_matrix[idx]            # [B,S,N] (N=10000) row gather
    d   = where(d == 0, 9999999.99, d)
    out = 1/d

Strategy (8 NeuronCores, SPMD single program):
  - Host computes idx (tiny: 6400 int lookups) and pads the 10000x10000 f32
    matrix to 10048 cols (dma_gather needs 256B-multiple rows); pad value 1.0.
  - Batch dim is sharded: core c handles batches [8c, 8c+8) = 800 (b,s) pairs.
  - On device, per chunk of 128 pairs: one gpsimd dma_gather pulls the 128
    indexed rows from HBM into SBUF (one row per partition), VectorE computes
    an approximate reciprocal (bit-trick seed + 2 Newton steps, existing
    RECIPROCAL_APPROX_FAST op), then one custom DVE op applies a final
    Newton step fused with the d==0 -> 1/BIG mask (~2 ULP total), and the
    result is DMA'd to the per-core output slab.
  - Host stacks the 8 per-core [800, 10000] outputs into [64, 100, 10000].

Everything value-dependent flows through input tensors, so the compiled
NEFF is input-independent and caches across calls.
"""

from contextlib import ExitStack

import numpy as np

import concourse.bacc as bacc
import concourse.mybir as mybir
import concourse.tile as tile
from concourse._compat import cdiv
from concourse.bass_utils import run_bass_kernel_spmd

# Problem shape (hardcoded per task contract).
N_POI = 10000
B, S = 64, 100
N_CORES = 8
PAIRS_PER_CORE = B * S // N_CORES  # 800
ELEM_PAD = 10048  # next multiple of 64 elems (256B) >= 10000
CHUNK = 128
BIG = 9999999.99
RBIG = float(np.float32(1.0) / np.float32(BIG))


def _register_recip_nr_masked():
    """Custom DVE op: out = select(in0 == 0, imm2, (s0 - in0*in1)*in1).

    One fused VectorE pass: the final Newton-Raphson refinement of an
    approximate reciprocal, with the zero-distance -> 1/BIG substitution.
    """
    from concourse import dve_ops
    from concourse.dve_spec import C0, C2, Spec, Src0, Src1, Zero, eq, select
    from concourse.dve_spec import lower as dve_lower
    from concourse.dve_uop import DveOpSpec

    name = "RECIP_NR_MASKED_V1"
    for o in dve_ops.OPS:
        if o.name == name:
            return o

    body = select(eq(Src0, Zero), C2, (C0 - Src0 * Src1) * Src1)

    def _ref(in0, in1, s0, s1, imm2):
        nr = (np.float32(s0) - in0.astype(np.float32) * in1.astype(np.float32)) * in1
        return np.where(in0 == 0.0, np.float32(imm2), nr).astype(np.float32)

    spec = Spec(body=body, reference=_ref)
    row = max(dve_ops._SUB_OPCODE_FOR_NAME.values()) + 1
    assert row < 0x20
    dve_ops._SUB_OPCODE_FOR_NAME[name] = row
    shas = {}
    for ver in ("v3",):
        s = DveOpSpec(name=name, opcode=row, uops=dve_lower(spec, ver=ver), rd1_en=True)
        shas[ver] = s.sha(ver)
    op = dve_ops.DveOp(name, spec, subdim=False, uops_sha=shas)
    dve_ops.OPS.append(op)
    dve_ops.CUSTOM_DVE_SPECS[name] = spec
    return op


def build_program(
    n_rows=N_POI,
    elem_pad=ELEM_PAD,
    out_cols=N_POI,
    n_pairs=PAIRS_PER_CORE,
    chunk=CHUNK,
):
    op = _register_recip_nr_masked()
    assert elem_pad % 64 == 0 and n_pairs % 16 == 0
    n_icols = cdiv(n_pairs, 16)

    nc = bacc.Bacc("TRN2", target_bir_lowering=False, debug=False)
    mat = nc.dram_tensor(
        "mat", [n_rows, elem_pad], mybir.dt.float32, kind="ExternalInput"
    ).ap()
    idx = nc.dram_tensor(
        "idx", [128, n_icols], mybir.dt.int16, kind="ExternalInput"
    ).ap()
    out = nc.dram_tensor(
        "out", [n_pairs, out_cols], mybir.dt.float32, kind="ExternalOutput"
    ).ap()

    with tile.TileContext(nc) as tc, ExitStack() as ctx:
        gpool = ctx.enter_context(tc.tile_pool(name="g", bufs=3))
        ypool = ctx.enter_context(tc.tile_pool(name="y", bufs=2))
        ipool = ctx.enter_context(tc.tile_pool(name="i", bufs=1))

        idx_t = ipool.tile([128, n_icols], mybir.dt.int16)
        nc.sync.dma_start(idx_t[:, :], idx)

        for c0 in range(0, n_pairs, chunk):
            n = min(chunk, n_pairs - c0)
            t = gpool.tile([128, 1, elem_pad], mybir.dt.float32, tag="t")
            nc.gpsimd.dma_gather(
                t[:, :, :],
                mat,
                idx_t[:, c0 // 16 : c0 // 16 + cdiv(n, 16)],
                n,
                n,
                elem_pad,
            )
            y = ypool.tile([128, elem_pad], mybir.dt.float32, tag="y")
            nc.vector.reciprocal_approx_fast(out=y[0:n, :], in_=t[0:n, 0, :])
            nc.vector._custom_dve(
                op,
                out=t[0:n, 0, 0:out_cols],
                in0=t[0:n, 0, 0:out_cols],
                in1=y[0:n, 0:out_cols],
                s0=2.0,
                imm2=RBIG,
            )
            nc.sync.dma_start(out[c0 : c0 + n, :], t[0:n, 0, 0:out_cols])

    nc.compile()
    return nc


def _wrap_idx(idx_flat: np.ndarray) -> np.ndarray:
    """[n] -> [128, n/16] int16 index-tile layout consumed by dma_gather
    (index i lives at [i % 16, i // 16], replicated over the 8 Q7 cores)."""
    n = idx_flat.shape[0]
    m = idx_flat.reshape(n // 16, 16).T.astype(np.int16)
    return np.tile(m, (8, 1))


def prepare_inputs(venueid2coor, inputs_poi, poi_distance_matrix):
    """Host-side prep: index lookup, matrix pad, per-core in_maps."""
    venueid2coor = np.asarray(venueid2coor)
    inputs_poi = np.asarray(inputs_poi)
    d = np.asarray(poi_distance_matrix, dtype=np.float32)

    idx = venueid2coor[inputs_poi].astype(np.int16)  # [B, S], values < N_POI
    mat = np.full((N_POI, ELEM_PAD), 1.0, dtype=np.float32)
    mat[:, :N_POI] = d

    bpc = B // N_CORES
    in_maps = [
        {"mat": mat, "idx": _wrap_idx(idx[c * bpc : (c + 1) * bpc].ravel())}
        for c in range(N_CORES)
    ]
    return in_maps


_PROGRAM_CACHE = {}


def _get_program():
    if "nc" not in _PROGRAM_CACHE:
        _PROGRAM_CACHE["nc"] = build_program()
    return _PROGRAM_CACHE["nc"]


def kernel(venueid2coor, inputs_poi, poi_distance_matrix) -> np.ndarray:
    nc = _get_program()
    in_maps = prepare_inputs(venueid2coor, inputs_poi, poi_distance_matrix)
    res = run_bass_kernel_spmd(nc, in_maps, list(range(N_CORES)))
    out = np.stack([res.results[c]["out"] for c in range(N_CORES)], axis=0)
    return out.reshape(B, S, N_POI).astype(np.float32)


# revision 2
# speedup vs baseline: 1013.1216x; 1013.1216x over previous
"""Trainium2 Bass kernel for nn_Attn_loc_distance (embedding lookup).

reference:
    idx = venueid2coor[inputs_poi]            # [B,S]   (B=64, S=100)
    d   = poi_distance_matrix[idx]            # [B,S,N] (N=10000) row gather
    d   = where(d == 0, 9999999.99, d)
    out = 1/d

Strategy (8 NeuronCores, SPMD single program):
  - Host computes idx (tiny: 6400 int lookups) and pads the 10000x10000 f32
    matrix to 10048 cols (dma_gather needs 256B-multiple rows); pad value 1.0.
  - Batch dim is sharded: core c handles batches [8c, 8c+8) = 800 (b,s) pairs.
  - On device, per chunk of 128 pairs: one gpsimd dma_gather pulls the 128
    indexed rows from HBM into SBUF (one row per partition), VectorE computes
    an approximate reciprocal (bit-trick seed + 2 Newton steps, existing
    RECIPROCAL_APPROX_FAST op), then one custom DVE op applies a final
    Newton step fused with the d==0 -> 1/BIG mask (~2 ULP total), and the
    result is DMA'd to the per-core output slab.
  - Host stacks the 8 per-core [800, 10000] outputs into [64, 100, 10000].

Everything value-dependent flows through input tensors, so the compiled
NEFF is input-independent and caches across calls.
"""

from contextlib import ExitStack

import numpy as np

import concourse.bacc as bacc
import concourse.mybir as mybir
import concourse.tile as tile
from concourse._compat import cdiv
from concourse.bass_utils import run_bass_kernel_spmd

# Problem shape (hardcoded per task contract).
N_POI = 10000
B, S = 64, 100
N_CORES = 8
PAIRS_PER_CORE = B * S // N_CORES  # 800
ELEM_PAD = 10048  # next multiple of 64 elems (256B) >= 10000
CHUNK = 128
BIG = 9999999.99
RBIG = float(np.float32(1.0) / np.float32(BIG))


def _register_recip_nr_masked():
    """Custom DVE op: out = select(in0 == 0, imm2, (s0 - in0*in1)*in1).

    One fused VectorE pass: the final Newton-Raphson refinement of an
    approximate reciprocal, with the zero-distance -> 1/BIG substitution.
    """
    from concourse import dve_ops
    from concourse.dve_spec import C0, C2, Spec, Src0, Src1, Zero, eq, select
    from concourse.dve_spec import lower as dve_lower
    from concourse.dve_uop import DveOpSpec

    name = "RECIP_NR_MASKED_V1"
    for o in dve_ops.OPS:
        if o.name == name:
            return o

    body = select(eq(Src0, Zero), C2, (C0 - Src0 * Src1) * Src1)

    def _ref(in0, in1, s0, s1, imm2):
        nr = (np.float32(s0) - in0.astype(np.float32) * in1.astype(np.float32)) * in1
        return np.where(in0 == 0.0, np.float32(imm2), nr).astype(np.float32)

    spec = Spec(body=body, reference=_ref)
    row = max(dve_ops._SUB_OPCODE_FOR_NAME.values()) + 1
    assert row < 0x20
    dve_ops._SUB_OPCODE_FOR_NAME[name] = row
    shas = {}
    for ver in ("v3",):
        s = DveOpSpec(name=name, opcode=row, uops=dve_lower(spec, ver=ver), rd1_en=True)
        shas[ver] = s.sha(ver)
    op = dve_ops.DveOp(name, spec, subdim=False, uops_sha=shas)
    dve_ops.OPS.append(op)
    dve_ops.CUSTOM_DVE_SPECS[name] = spec
    return op


def build_program(
    n_rows=N_POI,
    elem_pad=ELEM_PAD,
    out_cols=N_POI,
    n_pairs=PAIRS_PER_CORE,
    chunk=CHUNK,
    reps=1,
):
    """reps>1 repeats the body inside one NEFF (used only for timing: the
    marginal time per repetition is the device-side kernel time, free of
    dispatch overhead)."""
    op = _register_recip_nr_masked()
    assert elem_pad % 64 == 0 and n_pairs % 16 == 0
    n_icols = cdiv(n_pairs, 16)

    nc = bacc.Bacc("TRN2", target_bir_lowering=False, debug=False)
    mat = nc.dram_tensor(
        "mat", [n_rows, elem_pad], mybir.dt.float32, kind="ExternalInput"
    ).ap()
    idx = nc.dram_tensor(
        "idx", [128, n_icols], mybir.dt.int16, kind="ExternalInput"
    ).ap()
    out = nc.dram_tensor(
        "out", [n_pairs, out_cols], mybir.dt.float32, kind="ExternalOutput"
    ).ap()

    with tile.TileContext(nc) as tc, ExitStack() as ctx:
        gpool = ctx.enter_context(tc.tile_pool(name="g", bufs=3))
        ypool = ctx.enter_context(tc.tile_pool(name="y", bufs=2))
        ipool = ctx.enter_context(tc.tile_pool(name="i", bufs=1))

        idx_t = ipool.tile([128, n_icols], mybir.dt.int16)
        nc.sync.dma_start(idx_t[:, :], idx)

        for _rep in range(reps):
            for c0 in range(0, n_pairs, chunk):
                n = min(chunk, n_pairs - c0)
                t = gpool.tile([128, 1, elem_pad], mybir.dt.float32, tag="t")
                nc.gpsimd.dma_gather(
                    t[:, :, :],
                    mat,
                    idx_t[:, c0 // 16 : c0 // 16 + cdiv(n, 16)],
                    n,
                    n,
                    elem_pad,
                )
                y = ypool.tile([128, elem_pad], mybir.dt.float32, tag="y")
                nc.vector.reciprocal_approx_fast(out=y[0:n, :], in_=t[0:n, 0, :])
                nc.vector._custom_dve(
                    op,
                    out=t[0:n, 0, 0:out_cols],
                    in0=t[0:n, 0, 0:out_cols],
                    in1=y[0:n, 0:out_cols],
                    s0=2.0,
                    imm2=RBIG,
                )
                nc.sync.dma_start(out[c0 : c0 + n, :], t[0:n, 0, 0:out_cols])

    nc.compile()
    return nc


def _wrap_idx(idx_flat: np.ndarray) -> np.ndarray:
    """[n] -> [128, n/16] int16 index-tile layout consumed by dma_gather
    (index i lives at [i % 16, i // 16], replicated over the 8 Q7 cores)."""
    n = idx_flat.shape[0]
    m = idx_flat.reshape(n // 16, 16).T.astype(np.int16)
    return np.tile(m, (8, 1))


def prepare_inputs(venueid2coor, inputs_poi, poi_distance_matrix):
    """Host-side prep: index lookup, matrix pad, per-core in_maps."""
    venueid2coor = np.asarray(venueid2coor)
    inputs_poi = np.asarray(inputs_poi)
    d = np.asarray(poi_distance_matrix, dtype=np.float32)

    idx = venueid2coor[inputs_poi].astype(np.int16)  # [B, S], values < N_POI
    mat = np.full((N_POI, ELEM_PAD), 1.0, dtype=np.float32)
    mat[:, :N_POI] = d

    bpc = B // N_CORES
    in_maps = [
        {"mat": mat, "idx": _wrap_idx(idx[c * bpc : (c + 1) * bpc].ravel())}
        for c in range(N_CORES)
    ]
    return in_maps


_PROGRAM_CACHE = {}


def _get_program():
    if "nc" not in _PROGRAM_CACHE:
        _PROGRAM_CACHE["nc"] = build_program()
    return _PROGRAM_CACHE["nc"]


def kernel(venueid2coor, inputs_poi, poi_distance_matrix) -> np.ndarray:
    nc = _get_program()
    in_maps = prepare_inputs(venueid2coor, inputs_poi, poi_distance_matrix)
    res = run_bass_kernel_spmd(nc, in_maps, list(range(N_CORES)))
    out = np.stack([res.results[c]["out"] for c in range(N_CORES)], axis=0)
    return out.reshape(B, S, N_POI).astype(np.float32)
